# revision 37
# baseline (speedup 1.0000x reference)
"""Trainium2 Bass kernel: 3x3 valid conv (N=32, Cin=64, 128x128 -> Cout=128,
126x126) + bias, *0.5, then min over output channels.

Strategy (data-parallel over batch, 4 images per core on 8 cores):
- Conv in fp8e4m3 with DoubleRow matmuls: x is packed host-side as ONE fp8
  plane per image ([x | x shift-W] on partitions 0-63/64-127). The DoubleRow
  rhs is an overlapping strided AP (K-half dim stride = W), so half-0 reads
  the plane at column c and half-1 at c+W: one K=256 DoubleRow matmul per kw
  covers taps (0,kw),(1,kw),(2,kw) (the half-1 lower slot carries zero
  weights). The whole 3x3x64 conv is 3 matmuls per 512-position chunk.
- Weight-major rounds of G chunks amortize LDWEIGHTS (toolchain compiles
  with --enable-ldw-opt=false); kw order snakes across rounds so the
  framework's ldweights dedup saves a reload at each round boundary.
- Epilogue (v2): ScalarE drains PSUM->SBUF bf16 with bias+0.5 scale in
  PAIR-batched activations (one per two chunks, amortizing the ~352-cycle
  ACT overhead). The min over cout then uses a single fused DVE
  tensor_reduce(apply_transpose=True) per round: the DVE reshape front-end
  32x32-block-transposes the stream so the X-axis reduce collapses each
  32-cout partition group -> M[32i+a, F] = min over couts 32i..32i+31 of
  flat position 32F+a. This removes ALL per-chunk PE transposes (the old
  PE min path) and halves the DVE cost of the old two-pass
  StreamTranspose+reduce path. M is finalized per image by 4 small PE
  transposes + strided DVE reduces (min over the 4 cout groups), output in
  a permuted layout the host undoes with a reshape.
"""

import numpy as np
import ml_dtypes

N_CORES = 8
IMGS = 4  # images per core
H = W = 128
CIN = 64
COUT = 128
HW = H * W
XCOLS = HW + 512  # zero padding so the last chunk's shifted reads stay in-bounds
OUT = 126
NCHUNK = 32  # chunks of 512 flat output positions per image
G = 4  # chunks per weight-major round (must be even)
GP = 0  # number of rounds per image whose min-reduce goes to GPSIMD

_bf16 = ml_dtypes.bfloat16
_CACHE: dict = {}


def _gp_set(gp, nround):
    """Spread `gp` GPSIMD-offloaded rounds over the first nround-1 rounds
    (the last round is excluded: it has the ragged half-chunk)."""
    if not gp:
        return ()
    cand = list(range(nround - 1))
    step = len(cand) / gp
    return tuple(sorted({cand[min(int((i + 0.5) * step), len(cand) - 1)] for i in range(gp)}))


def _build_module(imgs=IMGS, repeats=1, dma_split=8, g=G, conv_only=False,
                  pe_fin=False, psa_bufs=4, no_reduce=False, gp=0,
                  ct_psum=False, drain=2, act_dummy=False, dve_drain=0):
    # drain granularity: chunks per ACT drain op (1, 2, or g).  drain=g uses
    # round-spanning PSUM tiles (g/2 banks each, psa_bufs of them).
    round_plan = []
    c = 0
    while c < NCHUNK:
        round_plan.append((c, min(g, NCHUNK - c)))
        c += g
    # last-chunk geometry: chunk 31 only has 256 valid positions
    import concourse.bass as bass
    import concourse.mybir as mybir
    import concourse.tile as tile
    from concourse import bacc
    from concourse.masks import make_identity

    f32 = mybir.dt.float32
    bf16 = mybir.dt.bfloat16
    f8 = mybir.dt.float8e4

    gp_set = _gp_set(gp, len(round_plan))

    nc = bacc.Bacc("TRN2", target_bir_lowering=False, debug=False)
    xf_d = nc.dram_tensor("xf1", [imgs, 128, XCOLS], f8, kind="ExternalInput")
    wf_d = nc.dram_tensor("wf", [128, 2, 3 * 128], f8, kind="ExternalInput")
    b_d = nc.dram_tensor("b", [128, 2], f32, kind="ExternalInput")
    o_d = nc.dram_tensor("out", [imgs, 4, 128, 32], f32, kind="ExternalOutput")
    og_d = None
    if gp_set:
        og_d = nc.dram_tensor(
            "outg", [imgs, len(gp_set), g * 512], f32, kind="ExternalOutput"
        )

    # staging map for the last image: quarter q (chunks 8q..8q+7) of M is
    # ready after the round that completes chunk 8q+7; stage its finalize one
    # round later for slack
    nround = len(round_plan)
    fin_stage = {}
    for q in range(3):
        done = 0
        for r, (c0, cg) in enumerate(round_plan):
            done += cg
            if done >= 8 * (q + 1):
                rr = min(r + 1, nround - 1)
                fin_stage.setdefault(rr, []).append(q)
                break

    import contextlib

    with tile.TileContext(nc) as tc:
        with contextlib.ExitStack() as stack:
            xp = stack.enter_context(tc.tile_pool(name="xp", bufs=3))
            wp = stack.enter_context(tc.tile_pool(name="wp", bufs=1))
            cp = stack.enter_context(tc.tile_pool(name="cp", bufs=3))
            mp = stack.enter_context(tc.tile_pool(name="mp", bufs=2))
            fp = stack.enter_context(tc.tile_pool(name="fp", bufs=2))
            psA = stack.enter_context(
                tc.tile_pool(name="psA", bufs=psa_bufs, space=bass.MemorySpace.PSUM)
            )
            psM = None
            if pe_fin:
                psM = stack.enter_context(
                    tc.tile_pool(name="psM", bufs=1, space=bass.MemorySpace.PSUM)
                )
            cps = None
            if ct_psum:
                cps = stack.enter_context(
                    tc.tile_pool(name="cps", bufs=1, space=bass.MemorySpace.PSUM)
                )
            wf = wp.tile([128, 2, 3 * 128], f8)
            nc.sync.dma_start(wf[:], wf_d[:])
            bt = wp.tile([128, 2], f32)
            nc.sync.dma_start(bt[:], b_d[:])
            idb = None
            if pe_fin:
                idb = wp.tile([128, 128], bf16)
                make_identity(nc, idb[:])
            dumt = None
            if act_dummy:
                dumt = wp.tile([128, 1024], bf16)
                nc.vector.memzero(dumt[:])

            import contextlib

            rep_ctx = tc.For_i(0, repeats) if repeats > 1 else contextlib.nullcontext()
            with rep_ctx:
              def finalize_q(MT, fin, M, n, q):
                # MT[p, q, f] = M[f, 128q + p]; then min over the 4
                # cout-groups i (f = 32i + a) -> fin[p, q, a]: result for
                # oh = 32q + p//4, ow = 32*(p%4) + a
                if pe_fin:
                    nc.tensor.transpose(
                        MT[:, q], M[:, q * 128 : (q + 1) * 128], idb[:]
                    )
                else:
                    # idle DMA xbar does the 128x128 transpose, freeing PE
                    # and the PSUM bank
                    nc.sync.dma_start_transpose(
                        MT[:, q], M[:, q * 128 : (q + 1) * 128]
                    )
                nc.vector.tensor_reduce(
                    fin[:, q],
                    MT[:, q].rearrange("p (i a) -> p a i", a=32),
                    axis=mybir.AxisListType.X,
                    op=mybir.AluOpType.min,
                )
                nc.sync.dma_start(o_d[n, q], fin[:, q])

              pending_fin = None

              for n in range(imgs):
                step = -(-XCOLS // dma_split)
                xft = xp.tile([128, XCOLS], f8, tag="xf", name=f"xf_{n}")
                for s in range(dma_split):
                    c0_, c1_ = s * step, min((s + 1) * step, XCOLS)
                    nc.sync.dma_start(xft[:, c0_:c1_], xf_d[n, :, c0_:c1_])

                M = MT = fin = None
                if not conv_only:
                    M = mp.tile([128, 512], bf16, tag="M", name=f"M_{n}")
                    if pe_fin:
                        MT = psM.tile([128, 4, 128], bf16, tag="mt", name=f"mt_{n}")
                    else:
                        MT = fp.tile([128, 4, 128], bf16, tag="mt", name=f"mt_{n}")
                    fin = fp.tile([128, 4, 32], f32, tag="fin", name=f"fin_{n}")
                    # cols for positions >= 16128 (cropped rows) never written
                    nc.vector.memzero(M[:, 504:512])

                import bass_rust as _br

                for r, (c0, cg) in enumerate(round_plan):
                    if drain >= g:
                        pst = psA.tile(
                            [128, cg, 512], f32, tag="ps", name=f"ps_{n}_{c0}"
                        )
                        pss = [
                            pst[:, 2 * p : 2 * p + 2] for p in range(cg // 2)
                        ]
                    else:
                        pst = None
                        pss = [
                            psA.tile(
                                [128, 2, 512], f32, tag="ps",
                                name=f"ps_{n}_{c0}_{p}",
                            )
                            for p in range(cg // 2)
                        ]
                    # weight-major with snaked kw order across rounds
                    kws = (0, 1, 2) if (r % 2 == 0) else (2, 1, 0)
                    for i, kw in enumerate(kws):
                        for gg in range(cg):
                            base = (c0 + gg) * 512
                            cw = 256 if c0 + gg == NCHUNK - 1 else 512
                            v = xft[:, base + kw : base + kw + cw]
                            rhs = _br.AP(
                                tensor=v.tensor,
                                offset=v.offset,
                                ap=[[v.ap[0][0], 128], [W, 2], [1, cw]],
                            )
                            nc.tensor.matmul(
                                pss[gg // 2][:, gg % 2, 0:cw],
                                wf[:, 0:2, kw * 128 : (kw + 1) * 128],
                                rhs,
                                start=(i == 0),
                                stop=(i == 2),
                                perf_mode=mybir.MatmulPerfMode.DoubleRow,
                            )
                    npair = cg // 2
                    if act_dummy:
                        # probe: tiny PSUM drains (free PSUM) + same-size
                        # SBUF->SBUF ACT traffic as real pair drains
                        ct = cp.tile(
                            [128, cg, 512], bf16, tag="ct", name=f"ct_{n}_{r}"
                        )
                        for p in range(npair):
                            nc.scalar.activation(
                                ct[:, 2 * p, 0:32],
                                pss[p][:, 0, 0:32],
                                mybir.ActivationFunctionType.Identity,
                            )
                            nc.scalar.activation(
                                ct[:, 2 * p : 2 * p + 2].rearrange(
                                    "p a b -> p (a b)"
                                ),
                                dumt[:],
                                mybir.ActivationFunctionType.Identity,
                                bias=bt[:, 0:1],
                                scale=0.5,
                            )
                        if r == len(round_plan) - 1 and n == imgs - 1:
                            drf = fp.tile([128, 32], f32, tag="drf", name="drf")
                            nc.any.tensor_copy(drf[:], ct[:, 0, 0:32])
                            nc.sync.dma_start(o_d[0, 0, :, :], drf[:])
                        continue
                    if conv_only:
                        # tiny per-pair drains keep PSUM consumed (an
                        # unconsumed variant wedges the exec unit at high R)
                        dr = cp.tile([128, npair, 32], f32, tag="ct",
                                     name=f"dr_{n}_{r}")
                        for p in range(npair):
                            nc.scalar.activation(
                                dr[:, p],
                                pss[p][:, 0, 0:32],
                                mybir.ActivationFunctionType.Identity,
                            )
                        if r == len(round_plan) - 1 and n == imgs - 1:
                            nc.sync.dma_start(o_d[0, 0, :, :], dr[:, 0])
                        continue
                    # ScalarE: PSUM -> SBUF bf16 with bias; granularity set
                    # by `drain` (chunks per ACT op).  The ragged tail (chunk
                    # 31 is half-width) shortens the last op.
                    gpr = r in gp_set
                    ct = (cps if ct_psum else cp).tile(
                        [128, cg, 512], bf16, tag="ct", name=f"ct_{n}_{r}"
                    )
                    valid = cg * 512 - (256 if c0 + cg == NCHUNK else 0)
                    ct_flat = ct[:].rearrange("p a b -> p (a b)")
                    bias_ap = bt[:, 1:2] if gpr else bt[:, 0:1]
                    scl = -0.5 if gpr else 0.5
                    if drain >= g:
                        nc.scalar.activation(
                            ct_flat[:, 0:valid],
                            pst[:].rearrange("p a b -> p (a b)")[:, 0:valid],
                            mybir.ActivationFunctionType.Identity,
                            bias=bias_ap,
                            scale=scl,
                        )
                    else:
                        step = drain * 512
                        for p in range(npair):
                            t_flat = pss[p][:].rearrange("p a b -> p (a b)")
                            use_dve = (r * npair + p) % 4 < dve_drain
                            for s in range(0, 1024, step):
                                lo = p * 1024 + s
                                pv = min(step, valid - lo)
                                if pv <= 0:
                                    break
                                if use_dve:
                                    # DVE drain: (psum * scl) + bias, own
                                    # PSUM read port
                                    nc.vector.tensor_scalar(
                                        ct_flat[:, lo : lo + pv],
                                        t_flat[:, s : s + pv],
                                        scl,
                                        bias_ap,
                                        op0=mybir.AluOpType.mult,
                                        op1=mybir.AluOpType.add,
                                    )
                                else:
                                    nc.scalar.activation(
                                        ct_flat[:, lo : lo + pv],
                                        t_flat[:, s : s + pv],
                                        mybir.ActivationFunctionType.Identity,
                                        bias=bias_ap,
                                        scale=scl,
                                    )
                    if no_reduce:
                        if r == len(round_plan) - 1 and n == imgs - 1:
                            drf = fp.tile([128, 32], f32, tag="drf", name="drf")
                            nc.any.tensor_copy(drf[:], ct[:, 0, 0:32])
                            nc.sync.dma_start(o_d[0, 0, :, :], drf[:])
                        continue
                    if gpr:
                        # GPSIMD: full min over all 128 couts in one shot via
                        # negated max; partition 0's row is the final
                        # (negated) result for these 2048 contiguous
                        # positions -> DMA straight out, no finalize needed
                        gout = fp.tile(
                            [128, cg * 512], f32, tag="go", name=f"go_{n}_{r}"
                        )
                        nc.gpsimd.partition_all_reduce(
                            gout[:],
                            ct[:].rearrange("p a b -> p (a b)"),
                            128,
                            bass.bass_isa.ReduceOp.max,
                        )
                        nc.sync.dma_start(
                            og_d[n, gp_set.index(r)], gout[0:1, :]
                        )
                    else:
                        # fused 32x32-block transpose + min over each 32-cout
                        # group, one DVE op per round:
                        # M[32i+a, 16*c0 + B] = min_b ct[32i+b, 32B+a]
                        nc.vector.tensor_reduce(
                            M[:, c0 * 16 : c0 * 16 + valid // 32],
                            ct[:]
                            .rearrange("p a b -> p (a b)")[:, 0:valid]
                            .rearrange("p (B b) -> p B b", b=32),
                            axis=mybir.AxisListType.X,
                            op=mybir.AluOpType.min,
                            apply_transpose=True,
                        )
                    if n == imgs - 1 and r in fin_stage:
                        for q in fin_stage[r]:
                            finalize_q(MT, fin, M, n, q)
                    if n > 0 and r == 1 and pending_fin is not None:
                        pM, pMT, pfin, pn = pending_fin
                        for q in range(4):
                            finalize_q(pMT, pfin, pM, pn, q)
                        pending_fin = None
                if conv_only or no_reduce:
                    continue
                if n == imgs - 1:
                    for q in range(4):
                        if q not in [x for v in fin_stage.values() for x in v]:
                            finalize_q(MT, fin, M, n, q)
                else:
                    pending_fin = (M, MT, fin, n)
    nc.compile()
    return nc


def _get_nc():
    if "nc" not in _CACHE:
        _CACHE["nc"] = _build_module(gp=GP)
    return _CACHE["nc"]


def _pack_inputs(x, weight, bias):
    x = np.asarray(x, np.float32)
    weight = np.asarray(weight, np.float32)
    bias = np.asarray(bias, np.float32)
    n_total = x.shape[0]

    # fp8 packing: x quantized from fp32; weights UNSCALED (the 0.5 folds
    # into the ScalarE activation) to stay in e4m3's normal range.
    _f8 = ml_dtypes.float8_e4m3
    xq = x.astype(_f8).reshape(n_total, CIN, HW)
    xf = np.zeros((n_total, 128, XCOLS), _f8)
    xf[:, 0:CIN, :HW] = xq
    xf[:, CIN:128, : HW - W] = xq[:, :, W:]  # x shift-W
    w_f8 = weight.astype(_f8)
    wfpack = np.zeros((128, 2, 3 * 128), _f8)
    for kw in range(3):
        wfpack[0:64, 0, kw * 128 : (kw + 1) * 128] = w_f8[:, :, 0, kw].T
        wfpack[64:128, 0, kw * 128 : (kw + 1) * 128] = w_f8[:, :, 1, kw].T
        wfpack[64:128, 1, kw * 128 : (kw + 1) * 128] = w_f8[:, :, 2, kw].T
    # ACT computes scale*in + bias, so fold SCALE=0.5 into the bias too.
    # col 0: +0.5b (min path), col 1: -0.5b (negated GPSIMD max path)
    bias_f = np.stack(
        [0.5 * bias, -0.5 * bias], axis=1
    ).astype(np.float32).reshape(128, 2)

    in_maps = []
    for core in range(N_CORES):
        in_maps.append(
            {
                "xf1": np.ascontiguousarray(xf[core * IMGS : (core + 1) * IMGS]),
                "wf": wfpack,
                "b": bias_f,
            }
        )
    return in_maps


def _unpack_out(o_m, o_g=None, g=G, gp_set=None):
    """o_m [n, 4, 128, 32]: value for oh = 32q + p//4, ow = 32*(p%4)+a at
    [n, q, p, a]; a straight reshape gives [oh, ow]. o_g [n, k, g*512] holds
    the (negated) GPSIMD rounds' results at flat positions r*g*512..."""
    n = o_m.shape[0]
    out = o_m.reshape(n, HW).copy()
    if o_g is not None and gp_set:
        for j, r in enumerate(gp_set):
            f0 = r * g * 512
            out[:, f0 : f0 + g * 512] = -o_g[:, j, :]
    return out.reshape(n, 128, 128)[:, :OUT, :OUT]


def _module_io(nc):
    import concourse.mybir as mybir

    part_name = nc.partition_id_tensor.name if nc.partition_id_tensor else None
    in_names, out_names, out_avals = [], [], []
    for alloc in nc.m.functions[0].allocations:
        if not isinstance(alloc, mybir.MemoryLocationSet):
            continue
        name = alloc.memorylocations[0].name
        if alloc.kind == "ExternalInput":
            if name != part_name:
                in_names.append(name)
        elif alloc.kind == "ExternalOutput":
            out_names.append(name)
            out_avals.append((tuple(alloc.tensor_shape), mybir.dt.np(alloc.dtype)))
    return in_names, out_names, out_avals, part_name


def _build_runner(nc):
    """Per-device jitted runners (no shard_map/mesh: pure data parallel)."""
    import jax
    from concourse.bass2jax import (
        _bass_exec_p,
        install_neuronx_cc_hook,
        partition_id_tensor,
    )

    install_neuronx_cc_hook()
    in_names, out_names, out_avals, part_name = _module_io(nc)
    all_names = tuple(in_names) + tuple(out_names)
    if part_name is not None:
        all_names = all_names + (part_name,)
    avals = tuple(jax.core.ShapedArray(shape, dtype) for shape, dtype in out_avals)

    def body(*args):
        extra = [partition_id_tensor()] if part_name else []
        outs = _bass_exec_p.bind(
            *args,
            *extra,
            out_avals=avals,
            in_names=all_names,
            out_names=tuple(out_names),
            lowering_input_output_aliases=(),
            sim_require_finite=True,
            sim_require_nnan=True,
            nc=nc,
        )
        return tuple(outs)

    fns = [jax.jit(body, device=d) for d in jax.devices()[:N_CORES]]
    return fns, in_names, out_names, out_avals


def _run_per_device(nc, in_maps):
    if "runner" not in _CACHE:
        _CACHE["runner"] = _build_runner(nc)
    fns, in_names, out_names, out_avals = _CACHE["runner"]
    zeros = [np.zeros(shape, dtype) for shape, dtype in out_avals]
    futs = []
    for core in range(N_CORES):
        args = [np.asarray(in_maps[core][n]) for n in in_names] + zeros
        futs.append(fns[core](*args))
    return [
        {name: np.asarray(f[i]) for i, name in enumerate(out_names)} for f in futs
    ]


def _run(x, weight, bias, trace=False):
    nc = _get_nc()
    in_maps = _pack_inputs(x, weight, bias)
    results = None
    last_exc = None
    for _attempt in range(3):
        try:
            results = _run_per_device(nc, in_maps)
            break
        except Exception as e:  # transient device errors: retry
            last_exc = e
    if results is None:
        # fall back to the spmd/shard_map path
        from concourse.bass_utils import run_bass_kernel_spmd

        try:
            res = run_bass_kernel_spmd(
                nc, in_maps, core_ids=list(range(N_CORES)), trace=False
            )
            results = res.results
        except Exception:
            raise last_exc
    nround = -(-NCHUNK // G)
    gp_set = _gp_set(GP, nround)
    out = np.empty((N_CORES * IMGS, 1, OUT, OUT), np.float32)
    for core in range(N_CORES):
        out[core * IMGS : (core + 1) * IMGS, 0] = _unpack_out(
            results[core]["out"], results[core].get("outg"), G, gp_set
        )
    return out, None


def kernel(x, weight, bias):
    out, _ = _run(x, weight, bias, trace=False)
    return out


# revision 47
# speedup vs baseline: 1.3009x; 1.3009x over previous
"""Trainium2 Bass kernel: 3x3 valid conv (N=32, Cin=64, 128x128 -> Cout=128,
126x126) + bias, *0.5, then min over output channels.

Strategy (data-parallel over batch, 4 images per core on 8 cores):
- Conv in fp8e4m3 with DoubleRow matmuls: x is packed host-side as ONE fp8
  plane per image ([x | x shift-W] on partitions 0-63/64-127). The DoubleRow
  rhs is an overlapping strided AP (K-half dim stride = W), so half-0 reads
  the plane at column c and half-1 at c+W: one K=256 DoubleRow matmul per kw
  covers taps (0,kw),(1,kw),(2,kw) (the half-1 lower slot carries zero
  weights). The whole 3x3x64 conv is 3 matmuls per 512-position chunk.
- Weight-major rounds of G chunks amortize LDWEIGHTS (toolchain compiles
  with --enable-ldw-opt=false); kw order snakes across rounds so the
  framework's ldweights dedup saves a reload at each round boundary.
- Epilogue (v2): ScalarE drains PSUM->SBUF bf16 with bias+0.5 scale in
  PAIR-batched activations (one per two chunks, amortizing the ~352-cycle
  ACT overhead). The min over cout then uses a single fused DVE
  tensor_reduce(apply_transpose=True) per round: the DVE reshape front-end
  32x32-block-transposes the stream so the X-axis reduce collapses each
  32-cout partition group -> M[32i+a, F] = min over couts 32i..32i+31 of
  flat position 32F+a. This removes ALL per-chunk PE transposes (the old
  PE min path) and halves the DVE cost of the old two-pass
  StreamTranspose+reduce path. M is finalized per image by 4 small PE
  transposes + strided DVE reduces (min over the 4 cout groups), output in
  a permuted layout the host undoes with a reshape.
"""

import numpy as np
import ml_dtypes

N_CORES = 8
IMGS = 4  # images per core
H = W = 128
CIN = 64
COUT = 128
HW = H * W
XCOLS = HW + 512  # zero padding so the last chunk's shifted reads stay in-bounds
OUT = 126
NCHUNK = 32  # chunks of 512 flat output positions per image
G = 4  # chunks per weight-major round (must be even)
GP = 0  # number of rounds per image whose min-reduce goes to GPSIMD
RAW = True  # raw-PSUM epilogue: fused reduce reads PSUM directly (no drains);
# couts are bias-sorted host-side and each 32-group's bias approximated by its
# mean, applied per-partition on M (adds ~0.0005 rel err; gate is 2e-2)

_bf16 = ml_dtypes.bfloat16
_CACHE: dict = {}


def _gp_set(gp, nround):
    """Spread `gp` GPSIMD-offloaded rounds over the first nround-1 rounds
    (the last round is excluded: it has the ragged half-chunk)."""
    if not gp:
        return ()
    cand = list(range(nround - 1))
    step = len(cand) / gp
    return tuple(sorted({cand[min(int((i + 0.5) * step), len(cand) - 1)] for i in range(gp)}))


def _build_module(imgs=IMGS, repeats=1, dma_split=8, g=G, conv_only=False,
                  pe_fin=False, psa_bufs=4, no_reduce=False, gp=0,
                  ct_psum=False, drain=2, act_dummy=False, dve_drain=0,
                  raw=None):
    # drain granularity: chunks per ACT drain op (1, 2, or g).  drain=g uses
    # round-spanning PSUM tiles (g/2 banks each, psa_bufs of them).
    if raw is None:
        raw = RAW
    if raw:
        drain, psa_bufs = g, 2  # round PSUM tiles, double-buffered
    round_plan = []
    c = 0
    while c < NCHUNK:
        round_plan.append((c, min(g, NCHUNK - c)))
        c += g
    # last-chunk geometry: chunk 31 only has 256 valid positions
    import concourse.bass as bass
    import concourse.mybir as mybir
    import concourse.tile as tile
    from concourse import bacc
    from concourse.masks import make_identity

    f32 = mybir.dt.float32
    bf16 = mybir.dt.bfloat16
    f8 = mybir.dt.float8e4

    gp_set = _gp_set(gp, len(round_plan))

    nc = bacc.Bacc("TRN2", target_bir_lowering=False, debug=False)
    xf_d = nc.dram_tensor("xf1", [imgs, 128, XCOLS], f8, kind="ExternalInput")
    wf_d = nc.dram_tensor("wf", [128, 2, 3 * 128], f8, kind="ExternalInput")
    b_d = nc.dram_tensor("b", [128, 3], f32, kind="ExternalInput")
    o_d = nc.dram_tensor("out", [imgs, 4, 128, 32], f32, kind="ExternalOutput")
    og_d = None
    if gp_set:
        og_d = nc.dram_tensor(
            "outg", [imgs, len(gp_set), g * 512], f32, kind="ExternalOutput"
        )

    # staging map for the last image: quarter q (chunks 8q..8q+7) of M is
    # ready after the round that completes chunk 8q+7; stage its finalize one
    # round later for slack
    nround = len(round_plan)
    fin_stage = {}
    for q in range(3):
        done = 0
        for r, (c0, cg) in enumerate(round_plan):
            done += cg
            if done >= 8 * (q + 1):
                rr = min(r + 1, nround - 1)
                fin_stage.setdefault(rr, []).append(q)
                break

    import contextlib

    with tile.TileContext(nc) as tc:
        with contextlib.ExitStack() as stack:
            xp = stack.enter_context(tc.tile_pool(name="xp", bufs=3))
            wp = stack.enter_context(tc.tile_pool(name="wp", bufs=1))
            cp = stack.enter_context(tc.tile_pool(name="cp", bufs=3))
            mp = stack.enter_context(tc.tile_pool(name="mp", bufs=2))
            fp = stack.enter_context(tc.tile_pool(name="fp", bufs=2))
            psA = stack.enter_context(
                tc.tile_pool(name="psA", bufs=psa_bufs, space=bass.MemorySpace.PSUM)
            )
            psM = None
            if pe_fin:
                psM = stack.enter_context(
                    tc.tile_pool(name="psM", bufs=1, space=bass.MemorySpace.PSUM)
                )
            cps = None
            if ct_psum:
                cps = stack.enter_context(
                    tc.tile_pool(name="cps", bufs=1, space=bass.MemorySpace.PSUM)
                )
            wf = wp.tile([128, 2, 3 * 128], f8)
            nc.sync.dma_start(wf[:], wf_d[:])
            bt = wp.tile([128, 3], f32)
            nc.sync.dma_start(bt[:], b_d[:])
            idb = None
            if pe_fin:
                idb = wp.tile([128, 128], bf16)
                make_identity(nc, idb[:])
            dumt = None
            if act_dummy:
                dumt = wp.tile([128, 1024], bf16)
                nc.vector.memzero(dumt[:])

            import contextlib

            rep_ctx = tc.For_i(0, repeats) if repeats > 1 else contextlib.nullcontext()
            with rep_ctx:
              def finalize_q(MT, fin, M, n, q, M2=None):
                # MT[p, q, f] = M[f, 128q + p]; then min over the 4
                # cout-groups i (f = 32i + a) -> fin[p, q, a]: result for
                # oh = 32q + p//4, ow = 32*(p%4) + a
                src = M[:, q * 128 : (q + 1) * 128]
                if raw:
                    # raw M holds biasless minima: apply the per-group bias
                    # approximation + SCALE here (per partition = per group)
                    nc.scalar.activation(
                        M2[:, q],
                        src,
                        mybir.ActivationFunctionType.Identity,
                        bias=bt[:, 2:3],
                        scale=0.5,
                    )
                    src = M2[:, q]
                if pe_fin:
                    nc.tensor.transpose(MT[:, q], src, idb[:])
                else:
                    # idle DMA xbar does the 128x128 transpose, freeing PE
                    # and the PSUM bank
                    nc.sync.dma_start_transpose(MT[:, q], src)
                nc.vector.tensor_reduce(
                    fin[:, q],
                    MT[:, q].rearrange("p (i a) -> p a i", a=32),
                    axis=mybir.AxisListType.X,
                    op=mybir.AluOpType.min,
                )
                nc.sync.dma_start(o_d[n, q], fin[:, q])

              pending_fin = None

              for n in range(imgs):
                step = -(-XCOLS // dma_split)
                xft = xp.tile([128, XCOLS], f8, tag="xf", name=f"xf_{n}")
                for s in range(dma_split):
                    c0_, c1_ = s * step, min((s + 1) * step, XCOLS)
                    nc.sync.dma_start(xft[:, c0_:c1_], xf_d[n, :, c0_:c1_])

                M = MT = fin = None
                M2 = None
                if not conv_only:
                    M = mp.tile([128, 512], bf16, tag="M", name=f"M_{n}")
                    if pe_fin:
                        MT = psM.tile([128, 4, 128], bf16, tag="mt", name=f"mt_{n}")
                    else:
                        MT = fp.tile([128, 4, 128], bf16, tag="mt", name=f"mt_{n}")
                    fin = fp.tile([128, 4, 32], f32, tag="fin", name=f"fin_{n}")
                    if raw:
                        M2 = fp.tile(
                            [128, 4, 128], bf16, tag="m2", name=f"m2_{n}"
                        )
                    # cols for positions >= 16128 (cropped rows) never written
                    nc.vector.memzero(M[:, 504:512])

                import bass_rust as _br

                for r, (c0, cg) in enumerate(round_plan):
                    if drain >= g:
                        pst = psA.tile(
                            [128, cg, 512], f32, tag="ps", name=f"ps_{n}_{c0}"
                        )
                        pss = [
                            pst[:, 2 * p : 2 * p + 2] for p in range(cg // 2)
                        ]
                    else:
                        pst = None
                        pss = [
                            psA.tile(
                                [128, 2, 512], f32, tag="ps",
                                name=f"ps_{n}_{c0}_{p}",
                            )
                            for p in range(cg // 2)
                        ]
                    # weight-major with snaked kw order across rounds
                    kws = (0, 1, 2) if (r % 2 == 0) else (2, 1, 0)
                    for i, kw in enumerate(kws):
                        for gg in range(cg):
                            base = (c0 + gg) * 512
                            cw = 256 if c0 + gg == NCHUNK - 1 else 512
                            v = xft[:, base + kw : base + kw + cw]
                            rhs = _br.AP(
                                tensor=v.tensor,
                                offset=v.offset,
                                ap=[[v.ap[0][0], 128], [W, 2], [1, cw]],
                            )
                            nc.tensor.matmul(
                                pss[gg // 2][:, gg % 2, 0:cw],
                                wf[:, 0:2, kw * 128 : (kw + 1) * 128],
                                rhs,
                                start=(i == 0),
                                stop=(i == 2),
                                perf_mode=mybir.MatmulPerfMode.DoubleRow,
                            )
                    npair = cg // 2
                    if act_dummy:
                        # probe: tiny PSUM drains (free PSUM) + same-size
                        # SBUF->SBUF ACT traffic as real pair drains
                        ct = cp.tile(
                            [128, cg, 512], bf16, tag="ct", name=f"ct_{n}_{r}"
                        )
                        for p in range(npair):
                            nc.scalar.activation(
                                ct[:, 2 * p, 0:32],
                                pss[p][:, 0, 0:32],
                                mybir.ActivationFunctionType.Identity,
                            )
                            nc.scalar.activation(
                                ct[:, 2 * p : 2 * p + 2].rearrange(
                                    "p a b -> p (a b)"
                                ),
                                dumt[:],
                                mybir.ActivationFunctionType.Identity,
                                bias=bt[:, 0:1],
                                scale=0.5,
                            )
                        if r == len(round_plan) - 1 and n == imgs - 1:
                            drf = fp.tile([128, 32], f32, tag="drf", name="drf")
                            nc.any.tensor_copy(drf[:], ct[:, 0, 0:32])
                            nc.sync.dma_start(o_d[0, 0, :, :], drf[:])
                        continue
                    if conv_only:
                        # tiny per-pair drains keep PSUM consumed (an
                        # unconsumed variant wedges the exec unit at high R)
                        dr = cp.tile([128, npair, 32], f32, tag="ct",
                                     name=f"dr_{n}_{r}")
                        for p in range(npair):
                            nc.scalar.activation(
                                dr[:, p],
                                pss[p][:, 0, 0:32],
                                mybir.ActivationFunctionType.Identity,
                            )
                        if r == len(round_plan) - 1 and n == imgs - 1:
                            nc.sync.dma_start(o_d[0, 0, :, :], dr[:, 0])
                        continue
                    valid = cg * 512 - (256 if c0 + cg == NCHUNK else 0)
                    if raw:
                        # fused transpose-reduce straight off the round's
                        # PSUM tile: M[32i+a, 16c0+B] = min over the i-th
                        # 32-cout (bias-sorted) group, raw (no bias/scale)
                        nc.vector.tensor_reduce(
                            M[:, c0 * 16 : c0 * 16 + valid // 32],
                            pst[:]
                            .rearrange("p a b -> p (a b)")[:, 0:valid]
                            .rearrange("p (B b) -> p B b", b=32),
                            axis=mybir.AxisListType.X,
                            op=mybir.AluOpType.min,
                            apply_transpose=True,
                        )
                        if n == imgs - 1 and r in fin_stage:
                            for q in fin_stage[r]:
                                finalize_q(MT, fin, M, n, q, M2)
                        if n > 0 and r == 1 and pending_fin is not None:
                            pM, pMT, pfin, pM2, pn = pending_fin
                            for q in range(4):
                                finalize_q(pMT, pfin, pM, pn, q, pM2)
                            pending_fin = None
                        continue
                    # ScalarE: PSUM -> SBUF bf16 with bias; granularity set
                    # by `drain` (chunks per ACT op).  The ragged tail (chunk
                    # 31 is half-width) shortens the last op.
                    gpr = r in gp_set
                    ct = (cps if ct_psum else cp).tile(
                        [128, cg, 512], bf16, tag="ct", name=f"ct_{n}_{r}"
                    )
                    ct_flat = ct[:].rearrange("p a b -> p (a b)")
                    bias_ap = bt[:, 1:2] if gpr else bt[:, 0:1]
                    scl = -0.5 if gpr else 0.5
                    if drain >= g:
                        nc.scalar.activation(
                            ct_flat[:, 0:valid],
                            pst[:].rearrange("p a b -> p (a b)")[:, 0:valid],
                            mybir.ActivationFunctionType.Identity,
                            bias=bias_ap,
                            scale=scl,
                        )
                    else:
                        step = drain * 512
                        for p in range(npair):
                            t_flat = pss[p][:].rearrange("p a b -> p (a b)")
                            use_dve = (r * npair + p) % 4 < dve_drain
                            for s in range(0, 1024, step):
                                lo = p * 1024 + s
                                pv = min(step, valid - lo)
                                if pv <= 0:
                                    break
                                if use_dve:
                                    # DVE drain: (psum * scl) + bias, own
                                    # PSUM read port
                                    nc.vector.tensor_scalar(
                                        ct_flat[:, lo : lo + pv],
                                        t_flat[:, s : s + pv],
                                        scl,
                                        bias_ap,
                                        op0=mybir.AluOpType.mult,
                                        op1=mybir.AluOpType.add,
                                    )
                                else:
                                    nc.scalar.activation(
                                        ct_flat[:, lo : lo + pv],
                                        t_flat[:, s : s + pv],
                                        mybir.ActivationFunctionType.Identity,
                                        bias=bias_ap,
                                        scale=scl,
                                    )
                    if no_reduce:
                        if r == len(round_plan) - 1 and n == imgs - 1:
                            drf = fp.tile([128, 32], f32, tag="drf", name="drf")
                            nc.any.tensor_copy(drf[:], ct[:, 0, 0:32])
                            nc.sync.dma_start(o_d[0, 0, :, :], drf[:])
                        continue
                    if gpr:
                        # GPSIMD: full min over all 128 couts in one shot via
                        # negated max; partition 0's row is the final
                        # (negated) result for these 2048 contiguous
                        # positions -> DMA straight out, no finalize needed
                        gout = fp.tile(
                            [128, cg * 512], f32, tag="go", name=f"go_{n}_{r}"
                        )
                        nc.gpsimd.partition_all_reduce(
                            gout[:],
                            ct[:].rearrange("p a b -> p (a b)"),
                            128,
                            bass.bass_isa.ReduceOp.max,
                        )
                        nc.sync.dma_start(
                            og_d[n, gp_set.index(r)], gout[0:1, :]
                        )
                    else:
                        # fused 32x32-block transpose + min over each 32-cout
                        # group, one DVE op per round:
                        # M[32i+a, 16*c0 + B] = min_b ct[32i+b, 32B+a]
                        nc.vector.tensor_reduce(
                            M[:, c0 * 16 : c0 * 16 + valid // 32],
                            ct[:]
                            .rearrange("p a b -> p (a b)")[:, 0:valid]
                            .rearrange("p (B b) -> p B b", b=32),
                            axis=mybir.AxisListType.X,
                            op=mybir.AluOpType.min,
                            apply_transpose=True,
                        )
                    if n == imgs - 1 and r in fin_stage:
                        for q in fin_stage[r]:
                            finalize_q(MT, fin, M, n, q, M2)
                    if n > 0 and r == 1 and pending_fin is not None:
                        pM, pMT, pfin, pM2, pn = pending_fin
                        for q in range(4):
                            finalize_q(pMT, pfin, pM, pn, q, pM2)
                        pending_fin = None
                if conv_only or no_reduce:
                    continue
                if n == imgs - 1:
                    for q in range(4):
                        if q not in [x for v in fin_stage.values() for x in v]:
                            finalize_q(MT, fin, M, n, q, M2)
                else:
                    pending_fin = (M, MT, fin, M2, n)
    nc.compile()
    return nc


def _get_nc():
    if "nc" not in _CACHE:
        _CACHE["nc"] = _build_module(gp=GP)
    return _CACHE["nc"]


def _pack_inputs(x, weight, bias):
    x = np.asarray(x, np.float32)
    weight = np.asarray(weight, np.float32)
    bias = np.asarray(bias, np.float32)
    n_total = x.shape[0]

    # fp8 packing: x quantized from fp32; weights UNSCALED (the 0.5 folds
    # into the epilogue) to stay in e4m3's normal range.  Output channels are
    # sorted by bias (the min over couts is permutation-invariant) so each
    # 32-cout group has a tight bias range; the raw epilogue approximates the
    # bias within a group by the group mean.
    _f8 = ml_dtypes.float8_e4m3
    xq = x.astype(_f8).reshape(n_total, CIN, HW)
    xf = np.zeros((n_total, 128, XCOLS), _f8)
    xf[:, 0:CIN, :HW] = xq
    xf[:, CIN:128, : HW - W] = xq[:, :, W:]  # x shift-W
    perm = np.argsort(bias, kind="stable")
    ws = weight[perm]
    bs = bias[perm]
    w_f8 = ws.astype(_f8)
    wfpack = np.zeros((128, 2, 3 * 128), _f8)
    for kw in range(3):
        wfpack[0:64, 0, kw * 128 : (kw + 1) * 128] = w_f8[:, :, 0, kw].T
        wfpack[64:128, 0, kw * 128 : (kw + 1) * 128] = w_f8[:, :, 1, kw].T
        wfpack[64:128, 1, kw * 128 : (kw + 1) * 128] = w_f8[:, :, 2, kw].T
    # bias cols (all pre-scaled by 0.5 since the epilogue applies scale*in +
    # bias): 0: +0.5b sorted (drain path), 1: -0.5b (negated GPSIMD path),
    # 2: 0.5 * group-mean bias (raw path, constant within each 32-group)
    bbar = np.repeat(bs.reshape(4, 32).mean(axis=1), 32)
    bias_f = np.stack(
        [0.5 * bs, -0.5 * bs, 0.5 * bbar], axis=1
    ).astype(np.float32).reshape(128, 3)

    in_maps = []
    for core in range(N_CORES):
        in_maps.append(
            {
                "xf1": np.ascontiguousarray(xf[core * IMGS : (core + 1) * IMGS]),
                "wf": wfpack,
                "b": bias_f,
            }
        )
    return in_maps


def _unpack_out(o_m, o_g=None, g=G, gp_set=None):
    """o_m [n, 4, 128, 32]: value for oh = 32q + p//4, ow = 32*(p%4)+a at
    [n, q, p, a]; a straight reshape gives [oh, ow]. o_g [n, k, g*512] holds
    the (negated) GPSIMD rounds' results at flat positions r*g*512..."""
    n = o_m.shape[0]
    out = o_m.reshape(n, HW).copy()
    if o_g is not None and gp_set:
        for j, r in enumerate(gp_set):
            f0 = r * g * 512
            out[:, f0 : f0 + g * 512] = -o_g[:, j, :]
    return out.reshape(n, 128, 128)[:, :OUT, :OUT]


def _module_io(nc):
    import concourse.mybir as mybir

    part_name = nc.partition_id_tensor.name if nc.partition_id_tensor else None
    in_names, out_names, out_avals = [], [], []
    for alloc in nc.m.functions[0].allocations:
        if not isinstance(alloc, mybir.MemoryLocationSet):
            continue
        name = alloc.memorylocations[0].name
        if alloc.kind == "ExternalInput":
            if name != part_name:
                in_names.append(name)
        elif alloc.kind == "ExternalOutput":
            out_names.append(name)
            out_avals.append((tuple(alloc.tensor_shape), mybir.dt.np(alloc.dtype)))
    return in_names, out_names, out_avals, part_name


def _build_runner(nc):
    """Per-device jitted runners (no shard_map/mesh: pure data parallel)."""
    import jax
    from concourse.bass2jax import (
        _bass_exec_p,
        install_neuronx_cc_hook,
        partition_id_tensor,
    )

    install_neuronx_cc_hook()
    in_names, out_names, out_avals, part_name = _module_io(nc)
    all_names = tuple(in_names) + tuple(out_names)
    if part_name is not None:
        all_names = all_names + (part_name,)
    avals = tuple(jax.core.ShapedArray(shape, dtype) for shape, dtype in out_avals)

    def body(*args):
        extra = [partition_id_tensor()] if part_name else []
        outs = _bass_exec_p.bind(
            *args,
            *extra,
            out_avals=avals,
            in_names=all_names,
            out_names=tuple(out_names),
            lowering_input_output_aliases=(),
            sim_require_finite=True,
            sim_require_nnan=True,
            nc=nc,
        )
        return tuple(outs)

    fns = [jax.jit(body, device=d) for d in jax.devices()[:N_CORES]]
    return fns, in_names, out_names, out_avals


def _run_per_device(nc, in_maps):
    if "runner" not in _CACHE:
        _CACHE["runner"] = _build_runner(nc)
    fns, in_names, out_names, out_avals = _CACHE["runner"]
    zeros = [np.zeros(shape, dtype) for shape, dtype in out_avals]
    futs = []
    for core in range(N_CORES):
        args = [np.asarray(in_maps[core][n]) for n in in_names] + zeros
        futs.append(fns[core](*args))
    return [
        {name: np.asarray(f[i]) for i, name in enumerate(out_names)} for f in futs
    ]


def _run(x, weight, bias, trace=False):
    nc = _get_nc()
    in_maps = _pack_inputs(x, weight, bias)
    results = None
    last_exc = None
    for _attempt in range(3):
        try:
            results = _run_per_device(nc, in_maps)
            break
        except Exception as e:  # transient device errors: retry
            last_exc = e
    if results is None:
        # fall back to the spmd/shard_map path
        from concourse.bass_utils import run_bass_kernel_spmd

        try:
            res = run_bass_kernel_spmd(
                nc, in_maps, core_ids=list(range(N_CORES)), trace=False
            )
            results = res.results
        except Exception:
            raise last_exc
    nround = -(-NCHUNK // G)
    gp_set = _gp_set(GP, nround)
    out = np.empty((N_CORES * IMGS, 1, OUT, OUT), np.float32)
    for core in range(N_CORES):
        out[core * IMGS : (core + 1) * IMGS, 0] = _unpack_out(
            results[core]["out"], results[core].get("outg"), G, gp_set
        )
    return out, None


def kernel(x, weight, bias):
    out, _ = _run(x, weight, bias, trace=False)
    return out


# revision 48
# speedup vs baseline: 1.3519x; 1.0392x over previous
"""Trainium2 Bass kernel: 3x3 valid conv (N=32, Cin=64, 128x128 -> Cout=128,
126x126) + bias, *0.5, then min over output channels.

Strategy (data-parallel over batch, 4 images per core on 8 cores):
- Conv in fp8e4m3 with DoubleRow matmuls: x is packed host-side as ONE fp8
  plane per image ([x | x shift-W] on partitions 0-63/64-127). The DoubleRow
  rhs is an overlapping strided AP (K-half dim stride = W), so half-0 reads
  the plane at column c and half-1 at c+W: one K=256 DoubleRow matmul per kw
  covers taps (0,kw),(1,kw),(2,kw) (the half-1 lower slot carries zero
  weights). The whole 3x3x64 conv is 3 matmuls per 512-position chunk.
- Weight-major rounds of G chunks amortize LDWEIGHTS (toolchain compiles
  with --enable-ldw-opt=false); kw order snakes across rounds so the
  framework's ldweights dedup saves a reload at each round boundary.
- Epilogue (v2): ScalarE drains PSUM->SBUF bf16 with bias+0.5 scale in
  PAIR-batched activations (one per two chunks, amortizing the ~352-cycle
  ACT overhead). The min over cout then uses a single fused DVE
  tensor_reduce(apply_transpose=True) per round: the DVE reshape front-end
  32x32-block-transposes the stream so the X-axis reduce collapses each
  32-cout partition group -> M[32i+a, F] = min over couts 32i..32i+31 of
  flat position 32F+a. This removes ALL per-chunk PE transposes (the old
  PE min path) and halves the DVE cost of the old two-pass
  StreamTranspose+reduce path. M is finalized per image by 4 small PE
  transposes + strided DVE reduces (min over the 4 cout groups), output in
  a permuted layout the host undoes with a reshape.
"""

import numpy as np
import ml_dtypes

N_CORES = 8
IMGS = 4  # images per core
H = W = 128
CIN = 64
COUT = 128
HW = H * W
XCOLS = HW + 512  # zero padding so the last chunk's shifted reads stay in-bounds
OUT = 126
NCHUNK = 32  # chunks of 512 flat output positions per image
G = 4  # chunks per weight-major round (must be even)
GP = 0  # number of rounds per image whose min-reduce goes to GPSIMD
RAW = False  # raw-PSUM epilogue (fused reduce reads PSUM directly, no ACT
# drains, bias-sorted couts with per-32-group mean-bias approximation).
# Measured 113.7us sustained vs 106.9us for the drain path: the PSUM-read
# contention tax follows whichever engine reads PSUM, and DVE reads slower
# (0.96 vs 1.2 GHz) with tighter PSUM buffering.  Kept for reference.

_bf16 = ml_dtypes.bfloat16
_CACHE: dict = {}


def _gp_set(gp, nround):
    """Spread `gp` GPSIMD-offloaded rounds over the first nround-1 rounds
    (the last round is excluded: it has the ragged half-chunk)."""
    if not gp:
        return ()
    cand = list(range(nround - 1))
    step = len(cand) / gp
    return tuple(sorted({cand[min(int((i + 0.5) * step), len(cand) - 1)] for i in range(gp)}))


def _build_module(imgs=IMGS, repeats=1, dma_split=8, g=G, conv_only=False,
                  pe_fin=False, psa_bufs=4, no_reduce=False, gp=0,
                  ct_psum=False, drain=2, act_dummy=False, dve_drain=0,
                  raw=None):
    # drain granularity: chunks per ACT drain op (1, 2, or g).  drain=g uses
    # round-spanning PSUM tiles (g/2 banks each, psa_bufs of them).
    if raw is None:
        raw = RAW
    if raw:
        drain, psa_bufs = g, 2  # round PSUM tiles, double-buffered
    round_plan = []
    c = 0
    while c < NCHUNK:
        round_plan.append((c, min(g, NCHUNK - c)))
        c += g
    # last-chunk geometry: chunk 31 only has 256 valid positions
    import concourse.bass as bass
    import concourse.mybir as mybir
    import concourse.tile as tile
    from concourse import bacc
    from concourse.masks import make_identity

    f32 = mybir.dt.float32
    bf16 = mybir.dt.bfloat16
    f8 = mybir.dt.float8e4

    gp_set = _gp_set(gp, len(round_plan))

    nc = bacc.Bacc("TRN2", target_bir_lowering=False, debug=False)
    xf_d = nc.dram_tensor("xf1", [imgs, 128, XCOLS], f8, kind="ExternalInput")
    wf_d = nc.dram_tensor("wf", [128, 2, 3 * 128], f8, kind="ExternalInput")
    b_d = nc.dram_tensor("b", [128, 3], f32, kind="ExternalInput")
    o_d = nc.dram_tensor("out", [imgs, 4, 128, 32], f32, kind="ExternalOutput")
    og_d = None
    if gp_set:
        og_d = nc.dram_tensor(
            "outg", [imgs, len(gp_set), g * 512], f32, kind="ExternalOutput"
        )

    # staging map for the last image: quarter q (chunks 8q..8q+7) of M is
    # ready after the round that completes chunk 8q+7; stage its finalize one
    # round later for slack
    nround = len(round_plan)
    fin_stage = {}
    for q in range(3):
        done = 0
        for r, (c0, cg) in enumerate(round_plan):
            done += cg
            if done >= 8 * (q + 1):
                rr = min(r + 1, nround - 1)
                fin_stage.setdefault(rr, []).append(q)
                break

    import contextlib

    with tile.TileContext(nc) as tc:
        with contextlib.ExitStack() as stack:
            xp = stack.enter_context(tc.tile_pool(name="xp", bufs=3))
            wp = stack.enter_context(tc.tile_pool(name="wp", bufs=1))
            cp = stack.enter_context(tc.tile_pool(name="cp", bufs=3))
            mp = stack.enter_context(tc.tile_pool(name="mp", bufs=2))
            fp = stack.enter_context(tc.tile_pool(name="fp", bufs=2))
            psA = stack.enter_context(
                tc.tile_pool(name="psA", bufs=psa_bufs, space=bass.MemorySpace.PSUM)
            )
            psM = None
            if pe_fin:
                psM = stack.enter_context(
                    tc.tile_pool(name="psM", bufs=1, space=bass.MemorySpace.PSUM)
                )
            cps = None
            if ct_psum:
                cps = stack.enter_context(
                    tc.tile_pool(name="cps", bufs=1, space=bass.MemorySpace.PSUM)
                )
            wf = wp.tile([128, 2, 3 * 128], f8)
            nc.sync.dma_start(wf[:], wf_d[:])
            bt = wp.tile([128, 3], f32)
            nc.sync.dma_start(bt[:], b_d[:])
            idb = None
            if pe_fin:
                idb = wp.tile([128, 128], bf16)
                make_identity(nc, idb[:])
            dumt = None
            if act_dummy:
                dumt = wp.tile([128, 1024], bf16)
                nc.vector.memzero(dumt[:])

            import contextlib

            rep_ctx = tc.For_i(0, repeats) if repeats > 1 else contextlib.nullcontext()
            with rep_ctx:
              def finalize_q(MT, fin, M, n, q, M2=None):
                # MT[p, q, f] = M[f, 128q + p]; then min over the 4
                # cout-groups i (f = 32i + a) -> fin[p, q, a]: result for
                # oh = 32q + p//4, ow = 32*(p%4) + a
                src = M[:, q * 128 : (q + 1) * 128]
                if raw:
                    # raw M holds biasless minima: apply the per-group bias
                    # approximation + SCALE here (per partition = per group)
                    nc.scalar.activation(
                        M2[:, q],
                        src,
                        mybir.ActivationFunctionType.Identity,
                        bias=bt[:, 2:3],
                        scale=0.5,
                    )
                    src = M2[:, q]
                if pe_fin:
                    nc.tensor.transpose(MT[:, q], src, idb[:])
                else:
                    # idle DMA xbar does the 128x128 transpose, freeing PE
                    # and the PSUM bank
                    nc.sync.dma_start_transpose(MT[:, q], src)
                nc.vector.tensor_reduce(
                    fin[:, q],
                    MT[:, q].rearrange("p (i a) -> p a i", a=32),
                    axis=mybir.AxisListType.X,
                    op=mybir.AluOpType.min,
                )
                nc.sync.dma_start(o_d[n, q], fin[:, q])

              pending_fin = None

              for n in range(imgs):
                step = -(-XCOLS // dma_split)
                xft = xp.tile([128, XCOLS], f8, tag="xf", name=f"xf_{n}")
                for s in range(dma_split):
                    c0_, c1_ = s * step, min((s + 1) * step, XCOLS)
                    nc.sync.dma_start(xft[:, c0_:c1_], xf_d[n, :, c0_:c1_])

                M = MT = fin = None
                M2 = None
                if not conv_only:
                    M = mp.tile([128, 512], bf16, tag="M", name=f"M_{n}")
                    if pe_fin:
                        MT = psM.tile([128, 4, 128], bf16, tag="mt", name=f"mt_{n}")
                    else:
                        MT = fp.tile([128, 4, 128], bf16, tag="mt", name=f"mt_{n}")
                    fin = fp.tile([128, 4, 32], f32, tag="fin", name=f"fin_{n}")
                    if raw:
                        M2 = fp.tile(
                            [128, 4, 128], bf16, tag="m2", name=f"m2_{n}"
                        )
                    # cols for positions >= 16128 (cropped rows) never written
                    nc.vector.memzero(M[:, 504:512])

                import bass_rust as _br

                for r, (c0, cg) in enumerate(round_plan):
                    if drain >= g:
                        pst = psA.tile(
                            [128, cg, 512], f32, tag="ps", name=f"ps_{n}_{c0}"
                        )
                        pss = [
                            pst[:, 2 * p : 2 * p + 2] for p in range(cg // 2)
                        ]
                    else:
                        pst = None
                        pss = [
                            psA.tile(
                                [128, 2, 512], f32, tag="ps",
                                name=f"ps_{n}_{c0}_{p}",
                            )
                            for p in range(cg // 2)
                        ]
                    # weight-major with snaked kw order across rounds
                    kws = (0, 1, 2) if (r % 2 == 0) else (2, 1, 0)
                    for i, kw in enumerate(kws):
                        for gg in range(cg):
                            base = (c0 + gg) * 512
                            cw = 256 if c0 + gg == NCHUNK - 1 else 512
                            v = xft[:, base + kw : base + kw + cw]
                            rhs = _br.AP(
                                tensor=v.tensor,
                                offset=v.offset,
                                ap=[[v.ap[0][0], 128], [W, 2], [1, cw]],
                            )
                            nc.tensor.matmul(
                                pss[gg // 2][:, gg % 2, 0:cw],
                                wf[:, 0:2, kw * 128 : (kw + 1) * 128],
                                rhs,
                                start=(i == 0),
                                stop=(i == 2),
                                perf_mode=mybir.MatmulPerfMode.DoubleRow,
                            )
                    npair = cg // 2
                    if act_dummy:
                        # probe: tiny PSUM drains (free PSUM) + same-size
                        # SBUF->SBUF ACT traffic as real pair drains
                        ct = cp.tile(
                            [128, cg, 512], bf16, tag="ct", name=f"ct_{n}_{r}"
                        )
                        for p in range(npair):
                            nc.scalar.activation(
                                ct[:, 2 * p, 0:32],
                                pss[p][:, 0, 0:32],
                                mybir.ActivationFunctionType.Identity,
                            )
                            nc.scalar.activation(
                                ct[:, 2 * p : 2 * p + 2].rearrange(
                                    "p a b -> p (a b)"
                                ),
                                dumt[:],
                                mybir.ActivationFunctionType.Identity,
                                bias=bt[:, 0:1],
                                scale=0.5,
                            )
                        if r == len(round_plan) - 1 and n == imgs - 1:
                            drf = fp.tile([128, 32], f32, tag="drf", name="drf")
                            nc.any.tensor_copy(drf[:], ct[:, 0, 0:32])
                            nc.sync.dma_start(o_d[0, 0, :, :], drf[:])
                        continue
                    if conv_only:
                        # tiny per-pair drains keep PSUM consumed (an
                        # unconsumed variant wedges the exec unit at high R)
                        dr = cp.tile([128, npair, 32], f32, tag="ct",
                                     name=f"dr_{n}_{r}")
                        for p in range(npair):
                            nc.scalar.activation(
                                dr[:, p],
                                pss[p][:, 0, 0:32],
                                mybir.ActivationFunctionType.Identity,
                            )
                        if r == len(round_plan) - 1 and n == imgs - 1:
                            nc.sync.dma_start(o_d[0, 0, :, :], dr[:, 0])
                        continue
                    valid = cg * 512 - (256 if c0 + cg == NCHUNK else 0)
                    if raw:
                        # fused transpose-reduce straight off the round's
                        # PSUM tile: M[32i+a, 16c0+B] = min over the i-th
                        # 32-cout (bias-sorted) group, raw (no bias/scale)
                        nc.vector.tensor_reduce(
                            M[:, c0 * 16 : c0 * 16 + valid // 32],
                            pst[:]
                            .rearrange("p a b -> p (a b)")[:, 0:valid]
                            .rearrange("p (B b) -> p B b", b=32),
                            axis=mybir.AxisListType.X,
                            op=mybir.AluOpType.min,
                            apply_transpose=True,
                        )
                        if n == imgs - 1 and r in fin_stage:
                            for q in fin_stage[r]:
                                finalize_q(MT, fin, M, n, q, M2)
                        if n > 0 and r == 1 and pending_fin is not None:
                            pM, pMT, pfin, pM2, pn = pending_fin
                            for q in range(4):
                                finalize_q(pMT, pfin, pM, pn, q, pM2)
                            pending_fin = None
                        continue
                    # ScalarE: PSUM -> SBUF bf16 with bias; granularity set
                    # by `drain` (chunks per ACT op).  The ragged tail (chunk
                    # 31 is half-width) shortens the last op.
                    gpr = r in gp_set
                    ct = (cps if ct_psum else cp).tile(
                        [128, cg, 512], bf16, tag="ct", name=f"ct_{n}_{r}"
                    )
                    ct_flat = ct[:].rearrange("p a b -> p (a b)")
                    bias_ap = bt[:, 1:2] if gpr else bt[:, 0:1]
                    scl = -0.5 if gpr else 0.5
                    if drain >= g:
                        nc.scalar.activation(
                            ct_flat[:, 0:valid],
                            pst[:].rearrange("p a b -> p (a b)")[:, 0:valid],
                            mybir.ActivationFunctionType.Identity,
                            bias=bias_ap,
                            scale=scl,
                        )
                    else:
                        step = drain * 512
                        for p in range(npair):
                            t_flat = pss[p][:].rearrange("p a b -> p (a b)")
                            use_dve = (r * npair + p) % 4 < dve_drain
                            for s in range(0, 1024, step):
                                lo = p * 1024 + s
                                pv = min(step, valid - lo)
                                if pv <= 0:
                                    break
                                if use_dve:
                                    # DVE drain: (psum * scl) + bias, own
                                    # PSUM read port
                                    nc.vector.tensor_scalar(
                                        ct_flat[:, lo : lo + pv],
                                        t_flat[:, s : s + pv],
                                        scl,
                                        bias_ap,
                                        op0=mybir.AluOpType.mult,
                                        op1=mybir.AluOpType.add,
                                    )
                                else:
                                    nc.scalar.activation(
                                        ct_flat[:, lo : lo + pv],
                                        t_flat[:, s : s + pv],
                                        mybir.ActivationFunctionType.Identity,
                                        bias=bias_ap,
                                        scale=scl,
                                    )
                    if no_reduce:
                        if r == len(round_plan) - 1 and n == imgs - 1:
                            drf = fp.tile([128, 32], f32, tag="drf", name="drf")
                            nc.any.tensor_copy(drf[:], ct[:, 0, 0:32])
                            nc.sync.dma_start(o_d[0, 0, :, :], drf[:])
                        continue
                    if gpr:
                        # GPSIMD: full min over all 128 couts in one shot via
                        # negated max; partition 0's row is the final
                        # (negated) result for these 2048 contiguous
                        # positions -> DMA straight out, no finalize needed
                        gout = fp.tile(
                            [128, cg * 512], f32, tag="go", name=f"go_{n}_{r}"
                        )
                        nc.gpsimd.partition_all_reduce(
                            gout[:],
                            ct[:].rearrange("p a b -> p (a b)"),
                            128,
                            bass.bass_isa.ReduceOp.max,
                        )
                        nc.sync.dma_start(
                            og_d[n, gp_set.index(r)], gout[0:1, :]
                        )
                    else:
                        # fused 32x32-block transpose + min over each 32-cout
                        # group, one DVE op per round:
                        # M[32i+a, 16*c0 + B] = min_b ct[32i+b, 32B+a]
                        nc.vector.tensor_reduce(
                            M[:, c0 * 16 : c0 * 16 + valid // 32],
                            ct[:]
                            .rearrange("p a b -> p (a b)")[:, 0:valid]
                            .rearrange("p (B b) -> p B b", b=32),
                            axis=mybir.AxisListType.X,
                            op=mybir.AluOpType.min,
                            apply_transpose=True,
                        )
                    if n == imgs - 1 and r in fin_stage:
                        for q in fin_stage[r]:
                            finalize_q(MT, fin, M, n, q, M2)
                    if n > 0 and r == 1 and pending_fin is not None:
                        pM, pMT, pfin, pM2, pn = pending_fin
                        for q in range(4):
                            finalize_q(pMT, pfin, pM, pn, q, pM2)
                        pending_fin = None
                if conv_only or no_reduce:
                    continue
                if n == imgs - 1:
                    for q in range(4):
                        if q not in [x for v in fin_stage.values() for x in v]:
                            finalize_q(MT, fin, M, n, q, M2)
                else:
                    pending_fin = (M, MT, fin, M2, n)
    nc.compile()
    return nc


def _get_nc():
    if "nc" not in _CACHE:
        _CACHE["nc"] = _build_module(gp=GP)
    return _CACHE["nc"]


def _pack_inputs(x, weight, bias):
    x = np.asarray(x, np.float32)
    weight = np.asarray(weight, np.float32)
    bias = np.asarray(bias, np.float32)
    n_total = x.shape[0]

    # fp8 packing: x quantized from fp32; weights UNSCALED (the 0.5 folds
    # into the epilogue) to stay in e4m3's normal range.  Output channels are
    # sorted by bias (the min over couts is permutation-invariant) so each
    # 32-cout group has a tight bias range; the raw epilogue approximates the
    # bias within a group by the group mean.
    _f8 = ml_dtypes.float8_e4m3
    xq = x.astype(_f8).reshape(n_total, CIN, HW)
    xf = np.zeros((n_total, 128, XCOLS), _f8)
    xf[:, 0:CIN, :HW] = xq
    xf[:, CIN:128, : HW - W] = xq[:, :, W:]  # x shift-W
    perm = np.argsort(bias, kind="stable")
    ws = weight[perm]
    bs = bias[perm]
    w_f8 = ws.astype(_f8)
    wfpack = np.zeros((128, 2, 3 * 128), _f8)
    for kw in range(3):
        wfpack[0:64, 0, kw * 128 : (kw + 1) * 128] = w_f8[:, :, 0, kw].T
        wfpack[64:128, 0, kw * 128 : (kw + 1) * 128] = w_f8[:, :, 1, kw].T
        wfpack[64:128, 1, kw * 128 : (kw + 1) * 128] = w_f8[:, :, 2, kw].T
    # bias cols (all pre-scaled by 0.5 since the epilogue applies scale*in +
    # bias): 0: +0.5b sorted (drain path), 1: -0.5b (negated GPSIMD path),
    # 2: 0.5 * group-mean bias (raw path, constant within each 32-group)
    bbar = np.repeat(bs.reshape(4, 32).mean(axis=1), 32)
    bias_f = np.stack(
        [0.5 * bs, -0.5 * bs, 0.5 * bbar], axis=1
    ).astype(np.float32).reshape(128, 3)

    in_maps = []
    for core in range(N_CORES):
        in_maps.append(
            {
                "xf1": np.ascontiguousarray(xf[core * IMGS : (core + 1) * IMGS]),
                "wf": wfpack,
                "b": bias_f,
            }
        )
    return in_maps


def _unpack_out(o_m, o_g=None, g=G, gp_set=None):
    """o_m [n, 4, 128, 32]: value for oh = 32q + p//4, ow = 32*(p%4)+a at
    [n, q, p, a]; a straight reshape gives [oh, ow]. o_g [n, k, g*512] holds
    the (negated) GPSIMD rounds' results at flat positions r*g*512..."""
    n = o_m.shape[0]
    out = o_m.reshape(n, HW).copy()
    if o_g is not None and gp_set:
        for j, r in enumerate(gp_set):
            f0 = r * g * 512
            out[:, f0 : f0 + g * 512] = -o_g[:, j, :]
    return out.reshape(n, 128, 128)[:, :OUT, :OUT]


def _module_io(nc):
    import concourse.mybir as mybir

    part_name = nc.partition_id_tensor.name if nc.partition_id_tensor else None
    in_names, out_names, out_avals = [], [], []
    for alloc in nc.m.functions[0].allocations:
        if not isinstance(alloc, mybir.MemoryLocationSet):
            continue
        name = alloc.memorylocations[0].name
        if alloc.kind == "ExternalInput":
            if name != part_name:
                in_names.append(name)
        elif alloc.kind == "ExternalOutput":
            out_names.append(name)
            out_avals.append((tuple(alloc.tensor_shape), mybir.dt.np(alloc.dtype)))
    return in_names, out_names, out_avals, part_name


def _build_runner(nc):
    """Per-device jitted runners (no shard_map/mesh: pure data parallel)."""
    import jax
    from concourse.bass2jax import (
        _bass_exec_p,
        install_neuronx_cc_hook,
        partition_id_tensor,
    )

    install_neuronx_cc_hook()
    in_names, out_names, out_avals, part_name = _module_io(nc)
    all_names = tuple(in_names) + tuple(out_names)
    if part_name is not None:
        all_names = all_names + (part_name,)
    avals = tuple(jax.core.ShapedArray(shape, dtype) for shape, dtype in out_avals)

    def body(*args):
        extra = [partition_id_tensor()] if part_name else []
        outs = _bass_exec_p.bind(
            *args,
            *extra,
            out_avals=avals,
            in_names=all_names,
            out_names=tuple(out_names),
            lowering_input_output_aliases=(),
            sim_require_finite=True,
            sim_require_nnan=True,
            nc=nc,
        )
        return tuple(outs)

    fns = [jax.jit(body, device=d) for d in jax.devices()[:N_CORES]]
    return fns, in_names, out_names, out_avals


def _run_per_device(nc, in_maps):
    if "runner" not in _CACHE:
        _CACHE["runner"] = _build_runner(nc)
    fns, in_names, out_names, out_avals = _CACHE["runner"]
    zeros = [np.zeros(shape, dtype) for shape, dtype in out_avals]
    futs = []
    for core in range(N_CORES):
        args = [np.asarray(in_maps[core][n]) for n in in_names] + zeros
        futs.append(fns[core](*args))
    return [
        {name: np.asarray(f[i]) for i, name in enumerate(out_names)} for f in futs
    ]


def _run(x, weight, bias, trace=False):
    nc = _get_nc()
    in_maps = _pack_inputs(x, weight, bias)
    results = None
    last_exc = None
    for _attempt in range(3):
        try:
            results = _run_per_device(nc, in_maps)
            break
        except Exception as e:  # transient device errors: retry
            last_exc = e
    if results is None:
        # fall back to the spmd/shard_map path
        from concourse.bass_utils import run_bass_kernel_spmd

        try:
            res = run_bass_kernel_spmd(
                nc, in_maps, core_ids=list(range(N_CORES)), trace=False
            )
            results = res.results
        except Exception:
            raise last_exc
    nround = -(-NCHUNK // G)
    gp_set = _gp_set(GP, nround)
    out = np.empty((N_CORES * IMGS, 1, OUT, OUT), np.float32)
    for core in range(N_CORES):
        out[core * IMGS : (core + 1) * IMGS, 0] = _unpack_out(
            results[core]["out"], results[core].get("outg"), G, gp_set
        )
    return out, None


def kernel(x, weight, bias):
    out, _ = _run(x, weight, bias, trace=False)
    return out
